# revision 3
# baseline (speedup 1.0000x reference)
"""Permutohedral lattice encoding + dense decode on 8 Trainium2 NeuronCores.

Data-parallel over points: each core processes N/8 = 32768 points end-to-end
(elevation matmul, lattice rounding/rank/barycentric, hash, table gather via
indirect DMA, weighted sum, 32x32 decode). Hash tables are replicated per
core in HBM. Self-contained: shapes/constants hardcoded from the problem
spec.
"""
import numpy as np

import concourse.bass as bass
import concourse.tile as tile
from concourse import mybir
import concourse.bacc as bacc
from concourse.bass_utils import run_bass_kernel_spmd
from contextlib import ExitStack

# ---- problem constants (hardcoded; kernel.py must be self-contained) ----
D = 3
L = 16
F = 2
LOG2_T = 18
T = 1 << LOG2_T
N = 262144
NCORES = 8
NPC = N // NCORES              # points per core = 32768
COARSEST, FINEST, POS_SCALE = 10.0, 1000.0, 1.0
PRIMES = np.array([1, 2654435761, 805459861], dtype=np.uint32)
RMAGIC = 12582912.0            # 1.5 * 2^23: round-to-nearest-even via add/sub

ST = 2048                      # points per supertile
NST = NPC // ST                # 16 supertiles
GP = ST // 128                 # point-groups (matmul tiles) per supertile = 16
PERPT = L * (D + 1)            # 64 values per point
FREE = GP * PERPT              # 1024 f32 per partition per supertile

FP = mybir.dt.float32
I32 = mybir.dt.int32


def _elev_matrix(d):
    E = np.zeros((d + 1, d), np.float32)
    E[0, :] = 1.0
    for i in range(1, d + 1):
        E[i, i:] = 1.0
        E[i, i - 1] = -float(i)
    return E


_E = _elev_matrix(D)
_growth = (FINEST / COARSEST) ** (1.0 / (L - 1))
_SCALES = (COARSEST * _growth ** np.arange(L)).astype(np.float32)
_INV_STD = (1.0 / np.sqrt(np.arange(1, D + 1) * np.arange(2, D + 2))).astype(np.float32)
_SF = (_SCALES[:, None] * POS_SCALE) * _INV_STD[None, :]               # [L, D]
# v = elevated/4 = x @ A + c ; A [3, 64] (le-major: l*4+e), c from shifts
_A = (np.einsum('ld,ed->dle', _SF, _E).reshape(D, PERPT) * 0.25).astype(np.float32)

# hash constants: key_i = 4*rvadj_i + (r - 4*[rank_i > 3-r]) ; contribution of
# coord i to the 18 low bits: (key_i * P_i) mod 2^18 computed via 9-bit limbs
_P18 = [int(p) % (1 << 18) for p in PRIMES]          # P mod 2^18
_Q = [(4 * int(p)) % (1 << 18) for p in PRIMES]      # (4P) mod 2^18 for the rvadj part
_QLO = [q & 511 for q in _Q]
_QMID = [(q >> 9) & 511 for q in _Q]
# per (coord, r): C0 = (r*P) mod 2^18 ; C1 = ((r-4)*P) mod 2^18
_C0 = [[(r * int(p)) % (1 << 18) for r in range(4)] for p in PRIMES]
_C1 = [[((r - 4) * int(p)) % (1 << 18) for r in range(4)] for p in PRIMES]

_CACHE = {}


def _build(npc, nst):
    """Build + compile the per-core Bass module. npc = points/core."""
    nc = bacc.Bacc("TRN2", target_bir_lowering=False, debug=False)
    xaugT = nc.dram_tensor("xaugT", [4, npc], FP, kind="ExternalInput").ap()
    tbl = nc.dram_tensor("tbl", [L * T, F], FP, kind="ExternalInput").ap()
    rhsA = nc.dram_tensor("rhsA", [4, PERPT], FP, kind="ExternalInput").ap()
    wt = nc.dram_tensor("wt", [32, 32], FP, kind="ExternalInput").ap()
    bcol = nc.dram_tensor("bcol", [32, 1], FP, kind="ExternalInput").ap()
    iotaE = nc.dram_tensor("iotaE", [128, FREE], FP, kind="ExternalInput").ap()
    ltT = nc.dram_tensor("ltT", [128, FREE], I32, kind="ExternalInput").ap()
    outT = nc.dram_tensor("outT", [32, npc], FP, kind="ExternalOutput").ap()

    with tile.TileContext(nc) as tc, ExitStack() as ctx:
        cst = ctx.enter_context(tc.tile_pool(name="cst", bufs=1))
        geo = ctx.enter_context(tc.tile_pool(name="geo", bufs=1))
        sm = ctx.enter_context(tc.tile_pool(name="sm", bufs=1))
        gat = ctx.enter_context(tc.tile_pool(name="gat", bufs=2))
        dec = ctx.enter_context(tc.tile_pool(name="dec", bufs=2))
        vps = ctx.enter_context(tc.tile_pool(name="vps", bufs=2, space="PSUM"))
        tps = ctx.enter_context(tc.tile_pool(name="tps", bufs=2, space="PSUM"))
        dps = ctx.enter_context(tc.tile_pool(name="dps", bufs=2, space="PSUM"))

        # constants loaded once
        rhs_t = cst.tile([4, PERPT], FP)
        nc.sync.dma_start(rhs_t[:], rhsA[:])
        wt_t = cst.tile([32, 32], FP)
        nc.sync.dma_start(wt_t[:], wt[:])
        b_t = cst.tile([32, 1], FP)
        nc.sync.dma_start(b_t[:], bcol[:])
        iota_t = cst.tile([128, FREE], FP)
        nc.sync.dma_start(iota_t[:], iotaE[:])
        lt_t = cst.tile([128, FREE], I32)
        nc.sync.dma_start(lt_t[:], ltT[:])
        cmag = cst.tile([128, 1], FP)
        nc.vector.memset(cmag[:], RMAGIC)
        ident = cst.tile([128, 128], FP)
        from concourse.masks import make_identity
        make_identity(nc, ident[:])

        for st in range(nst):
            # ---- elevate: v = xaug @ [A; c] (psum [128, FREE], le-major) ----
            xa = geo.tile([4, ST], FP, tag="xa")
            nc.sync.dma_start(xa[:], xaugT[:, st * ST:(st + 1) * ST])
            vp = vps.tile([128, FREE], FP, tag="vp")
            for g in range(GP):
                nc.tensor.matmul(
                    vp[:, g * PERPT:(g + 1) * PERPT],
                    lhsT=xa[:, g * 128:(g + 1) * 128],
                    rhs=rhs_t[:],
                    start=True, stop=True,
                )
            # views [128, GP, L, 4]
            def v4(t, d_last=4):
                return t[:].rearrange("p (g l e) -> p g l e", g=GP, e=d_last)

            # ---- round / delta / s ----
            rc = geo.tile([128, FREE], FP, tag="rc")
            nc.scalar.activation(rc[:], vp[:], mybir.ActivationFunctionType.Identity,
                                 bias=cmag[:])
            rv = geo.tile([128, FREE], FP, tag="rv")
            nc.vector.tensor_scalar(rv[:], rc[:], RMAGIC, None,
                                    op0=mybir.AluOpType.subtract)
            dv = geo.tile([128, FREE], FP, tag="dv")
            nc.vector.tensor_tensor(dv[:], vp[:], rv[:], op=mybir.AluOpType.subtract)
            s_t = geo.tile([128, FREE // 4], FP, tag="s")
            nc.vector.tensor_reduce(s_t[:], v4(rv), mybir.AxisListType.X,
                                    mybir.AluOpType.add)

            # ---- rank: init iota+s, pairwise greater, scatter add/sub ----
            rank = geo.tile([128, FREE], FP, tag="rank")
            nc.vector.tensor_tensor(
                rank[:], iota_t[:],
                s_t[:].rearrange("p (g l) -> p g l", g=GP).to_broadcast([128, GP, L, 4]),
                op=mybir.AluOpType.add)
            dvv = v4(dv)
            rkv = v4(rank)
            for dist in (1, 2, 3):
                gt = geo.tile([128, GP * L * (4 - dist)], FP, tag=f"gcmp{dist}")
                gv = gt[:].rearrange("p (g l e) -> p g l e", g=GP, e=4 - dist)
                nc.vector.tensor_tensor(gv, dvv[:, :, :, dist:4], dvv[:, :, :, 0:4 - dist],
                                        op=mybir.AluOpType.is_gt)
                nc.vector.tensor_tensor(rkv[:, :, :, 0:4 - dist], rkv[:, :, :, 0:4 - dist],
                                        gv, op=mybir.AluOpType.add)
                nc.vector.tensor_tensor(rkv[:, :, :, dist:4], rkv[:, :, :, dist:4],
                                        gv, op=mybir.AluOpType.subtract)

            # ---- wrap: rank_w = rank & 3 ; wr = (rank_w - rank) >> 2 ----
            rank_i = geo.tile([128, FREE], I32, tag="rank_i")
            nc.vector.tensor_copy(rank_i[:], rank[:])
            rank_w = geo.tile([128, FREE], I32, tag="rank_w")
            nc.vector.tensor_scalar(rank_w[:], rank_i[:], 3, None,
                                    op0=mybir.AluOpType.bitwise_and)
            wr = geo.tile([128, FREE], I32, tag="wr")
            nc.vector.tensor_tensor(wr[:], rank_w[:], rank_i[:],
                                    op=mybir.AluOpType.subtract)
            nc.vector.tensor_scalar(wr[:], wr[:], 2, None,
                                    op0=mybir.AluOpType.arith_shift_right)
            rv_i = geo.tile([128, FREE], I32, tag="rv_i")
            nc.vector.tensor_copy(rv_i[:], rv[:])
            rvadj = geo.tile([128, FREE], I32, tag="rvadj")
            nc.vector.tensor_tensor(rvadj[:], rv_i[:], wr[:], op=mybir.AluOpType.add)
            dvw = geo.tile([128, FREE], FP, tag="dvw")
            nc.vector.tensor_tensor(dvw[:], dv[:], wr[:], op=mybir.AluOpType.subtract)

            # ---- thresholds on wrapped rank (f32 0/1) ----
            gts = []
            for thr in (0, 1, 2):
                g = geo.tile([128, FREE], FP, tag=f"gt{thr}")
                nc.vector.tensor_scalar(g[:], rank_w[:], thr, None,
                                        op0=mybir.AluOpType.is_gt)
                gts.append(g)

            # ---- barycentric weights [g, l, r] via threshold-masked sums ----
            q = FREE // 4
            Gs = []
            for thr in (0, 1, 2):
                m = sm.tile([128, FREE], FP, tag=f"gm{thr}")
                nc.vector.tensor_tensor(m[:], dvw[:], gts[thr][:],
                                        op=mybir.AluOpType.mult)
                Gr = sm.tile([128, q], FP, tag=f"G{thr}")
                nc.vector.tensor_reduce(Gr[:], v4(m), mybir.AxisListType.X,
                                        mybir.AluOpType.add)
                Gs.append(Gr)
            Sdv = sm.tile([128, q], FP, tag="Sdv")
            nc.vector.tensor_reduce(Sdv[:], v4(dvw), mybir.AxisListType.X,
                                    mybir.AluOpType.add)
            G0, G1, G2 = Gs
            bary = sm.tile([128, FREE], FP, tag="bary")
            bv = v4(bary)
            t1 = sm.tile([128, q], FP, tag="t1")
            # bary0 = 1 + G2 - Sdv + G0
            nc.vector.tensor_tensor(t1[:], G0[:], Sdv[:], op=mybir.AluOpType.subtract)
            nc.vector.tensor_tensor(t1[:], t1[:], G2[:], op=mybir.AluOpType.add)
            nc.vector.tensor_scalar(bv[:, :, :, 0:1], t1[:], 1.0, None,
                                    op0=mybir.AluOpType.add)
            # bary1 = G1 - 2*G2 ; bary2 = G0 - 2*G1 + G2 ; bary3 = Sdv - 2*G0 + G1
            for (slot, a, bq, c2) in ((1, G1, G2, None), (2, G0, G1, G2), (3, Sdv, G0, G1)):
                tt = sm.tile([128, q], FP, tag=f"tb{slot}")
                nc.vector.tensor_scalar(tt[:], bq[:], 2.0, None,
                                        op0=mybir.AluOpType.mult)
                nc.vector.tensor_tensor(tt[:], a[:], tt[:], op=mybir.AluOpType.subtract)
                if c2 is not None:
                    nc.vector.tensor_tensor(tt[:], tt[:], c2[:], op=mybir.AluOpType.add)
                nc.vector.tensor_copy(bv[:, :, :, slot:slot + 1], tt[:])

            # ---- hash -> idx [g, l, r] int32 ----
            # per coordinate contribution c_i[g, l, r] (18 low bits + garbage hi)
            cts = []
            for ci in range(3):
                rva_s = v4(rvadj)[:, :, :, ci:ci + 1]      # [128, GP, L, 1]
                ct = sm.tile([128, FREE], I32, tag=f"hc{ci}")
                ctv = ct[:].rearrange("p (g l r) -> p g l r", g=GP, r=4)
                if ci == 0:
                    base = sm.tile([128, q], I32, tag="hb0")
                    nc.vector.tensor_scalar(
                        base[:], rva_s, 2, None,
                        op0=mybir.AluOpType.logical_shift_left)
                else:
                    m1 = sm.tile([128, q], FP, tag=f"hm1_{ci}")
                    nc.vector.tensor_scalar(
                        m1[:], rva_s,
                        float(_QLO[ci]), None, op0=mybir.AluOpType.mult)
                    m2 = sm.tile([128, q], FP, tag=f"hm2_{ci}")
                    nc.vector.tensor_scalar(
                        m2[:], rva_s,
                        float(_QMID[ci]), None, op0=mybir.AluOpType.mult)
                    m1i = sm.tile([128, q], I32, tag=f"hm1i_{ci}")
                    nc.vector.tensor_copy(m1i[:], m1[:])
                    m2i = sm.tile([128, q], I32, tag=f"hm2i_{ci}")
                    nc.vector.tensor_copy(m2i[:], m2[:])
                    nc.vector.tensor_scalar(m2i[:], m2i[:], 511, None,
                                            op0=mybir.AluOpType.bitwise_and)
                    nc.vector.tensor_scalar(m2i[:], m2i[:], 9, None,
                                            op0=mybir.AluOpType.logical_shift_left)
                    base = sm.tile([128, q], I32, tag=f"hb{ci}")
                    nc.vector.tensor_tensor(base[:], m1i[:], m2i[:],
                                            op=mybir.AluOpType.add)
                basev = base[:]
                nc.vector.tensor_copy(ctv[:, :, :, 0:1], basev)
                for r in (1, 2, 3):
                    gtx = gts[3 - r]                        # [rank > 3-r]
                    c0 = float(_C0[ci][r])
                    dc = float(_C1[ci][r] - _C0[ci][r])
                    trt = sm.tile([128, q], FP, tag=f"ht{ci}")
                    nc.vector.tensor_scalar(
                        trt[:], v4(gtx)[:, :, :, ci:ci + 1],
                        dc, c0, op0=mybir.AluOpType.mult, op1=mybir.AluOpType.add)
                    nc.vector.tensor_tensor(
                        ctv[:, :, :, r:r + 1], trt[:], basev,
                        op=mybir.AluOpType.add)
                cts.append(ct)
            idx = sm.tile([128, FREE], I32, tag="idx")
            nc.vector.tensor_tensor(idx[:], cts[0][:], cts[1][:],
                                    op=mybir.AluOpType.bitwise_xor)
            nc.vector.tensor_tensor(idx[:], idx[:], cts[2][:],
                                    op=mybir.AluOpType.bitwise_xor)
            nc.vector.tensor_scalar(idx[:], idx[:], T - 1, None,
                                    op0=mybir.AluOpType.bitwise_and)
            nc.vector.tensor_tensor(idx[:], idx[:], lt_t[:], op=mybir.AluOpType.add)

            # ---- gather: one indirect DMA per idx column (128 lookups) ----
            feats = gat.tile([128, FREE * F], FP, tag="feats")
            for c in range(FREE):
                nc.gpsimd.indirect_dma_start(
                    out=feats[:, c * F:(c + 1) * F],
                    out_offset=None, in_=tbl[:],
                    in_offset=bass.IndirectOffsetOnAxis(ap=idx[:, c:c + 1], axis=0),
                )

            # ---- weighted sum over the 4 vertices -> enc [g, l, f] ----
            wf = gat.tile([128, FREE * F], FP, tag="wf")
            nc.vector.tensor_tensor(
                wf[:].rearrange("p (gl r f) -> p gl r f", r=4, f=F),
                feats[:].rearrange("p (gl r f) -> p gl r f", r=4, f=F),
                bary[:].rearrange("p (gl r) -> p gl r", r=4).to_broadcast(
                    [128, GP * L, 4, F]),
                op=mybir.AluOpType.mult)
            enc = dec.tile([128, GP * L * F], FP, tag="enc")
            # reduce over r (middle): AP with r as innermost
            wfr = bass.AP(
                tensor=wf[:].tensor, offset=wf[:].offset,
                ap=[wf[:].ap[0], [8, GP * L], [1, F], [2, 4]])
            nc.vector.tensor_reduce(
                enc[:].rearrange("p (gl f) -> p gl f", f=F), wfr,
                mybir.AxisListType.X, mybir.AluOpType.add)

            # ---- decode: out = enc @ W.T + b, emitted transposed ----
            etp = dec.tile([32, ST], FP, tag="etp")
            for g in range(GP):
                tp = tps.tile([32, 128], FP, tag="tp")
                nc.tensor.transpose(
                    out=tp[:], in_=enc[:, g * 32:(g + 1) * 32],
                    identity=ident[:])
                nc.scalar.copy(etp[:, g * 128:(g + 1) * 128], tp[:])
            for h in range(ST // 512):
                dp = dps.tile([32, 512], FP, tag="dp")
                nc.tensor.matmul(dp[:], lhsT=wt_t[:],
                                 rhs=etp[:, h * 512:(h + 1) * 512],
                                 start=True, stop=True)
                ot = dec.tile([32, 512], FP, tag="ot")
                nc.scalar.activation(ot[:], dp[:],
                                     mybir.ActivationFunctionType.Identity,
                                     bias=b_t[:])
                nc.sync.dma_start(
                    outT[:, st * ST + h * 512: st * ST + (h + 1) * 512], ot[:])

    nc.compile()
    return nc


def _prep_consts():
    iota = np.tile(np.arange(4, dtype=np.float32), FREE // 4)       # e pattern
    iota_t = np.broadcast_to(iota, (128, FREE)).copy()
    lt = np.tile(np.repeat(np.arange(L, dtype=np.int64) * T, 4), GP)
    lt_t = np.broadcast_to(lt.astype(np.int32), (128, FREE)).copy()
    return iota_t, lt_t


def kernel(x, table, shifts, W, b):
    x = np.asarray(x, np.float32)
    table = np.asarray(table, np.float32)
    shifts = np.asarray(shifts, np.float32)
    W = np.asarray(W, np.float32)
    b = np.asarray(b, np.float32)

    key = (NPC, NST)
    if key not in _CACHE:
        _CACHE[key] = _build(NPC, NST)
    nc = _CACHE[key]

    c = (np.einsum('ld,ld,ed->le', shifts, _SF, _E).reshape(PERPT) * 0.25)
    rhsA = np.concatenate([_A, c[None, :].astype(np.float32)], 0).astype(np.float32)
    tbl = np.ascontiguousarray(table.reshape(L * T, F))
    wt = np.ascontiguousarray(W.T)
    bcol = np.ascontiguousarray(b.reshape(32, 1))
    iota_t, lt_t = _prep_consts()

    in_maps = []
    for k in range(NCORES):
        xs = x[k * NPC:(k + 1) * NPC]
        # point (g-tile t, partition p) = global point t*128+p within the core
        xaugT = np.concatenate([xs.T, np.ones((1, NPC), np.float32)], 0)
        xaugT = np.ascontiguousarray(xaugT)
        in_maps.append({
            "xaugT": xaugT, "tbl": tbl, "rhsA": rhsA, "wt": wt,
            "bcol": bcol, "iotaE": iota_t, "ltT": lt_t,
        })

    import os
    trace = bool(os.environ.get("KERNEL_TRACE"))
    res = run_bass_kernel_spmd(nc, in_maps, core_ids=list(range(NCORES)),
                               trace=trace)
    global LAST_EXEC_NS
    LAST_EXEC_NS = res.exec_time_ns
    out = np.empty((N, 32), np.float32)
    for k in range(NCORES):
        out[k * NPC:(k + 1) * NPC] = res.results[k]["outT"].T
    return out


LAST_EXEC_NS = None
